# revision 21
# baseline (speedup 1.0000x reference)
"""Criss-cross attention block (CCNet) Bass/Tile kernel for Trainium2.

Shapes (hardcoded): B=8, C=256, H=W=128, CR=32. Data-parallel over batch:
core b processes image b. Full inputs in, full output out.

Per-core plan (v5):
  P1   : stream x (f32->bf16 cast DMA on gpsimd), QKV projections with
         paired weight loads, ONE ACT evac [96,512] per chunk -> tA
         (K@0, Q@32, V@64 h-major). Q replica -> tB[0:32] via one
         SBUF->SBUF DMA per quarter (sync queue).
  vtsR : row-mode V^T via DMA-XBAR transpose per quarter:
         [48,4096] (V + ones rows tA[96:112]) -> vts_row[:,q] dense;
         XBAR maps logical row f to partition f%128 = w.
  row  : energies on T0 (K@0 x Q@0), exp on ACT, apply (V^T @ expe)
         evac via DVE into zin[0:33] (h-major; Zr row at 32).
  vtsC : col-mode V^T via PE transposes (strided V columns), packed
         4/psum, evacs alternate DVE/ACT, interleaved with col loop.
  col  : apply evac: contiguous CAST (DVE/ACT) to zcs scratch + strided
         second hop (gpsimd/DVE) into zin[64:97] h-major (Zc at 96).
  Z    : Zr=zin[32], Zc=zin[96] both h-major; one DRAM roundtrip per Z
         row to reshape [1,HW]->[128,128]; r = 1/(Zr+Zc) -> rscr;
         broadcast per h-chunk into rb=tA[0:97] (sync queue).
  P5   : prenorm zin *= rb (chunked, DVE/gpsimd split), ONE 96-deep
         matmul per psum tile (wzT96 = [WzT;0...;WzT]), residual add vs
         x_bf, bf16 staging, one cast (bf16->f32) output DMA per chunk.
"""
import sys

sys.path.insert(0, "/opt/trn_rl_repo")

import numpy as np
import ml_dtypes

import concourse.bass as bass
import concourse.mybir as mybir
from concourse import bacc, tile
from concourse.bass_utils import run_bass_kernel_spmd

B, C, H, W, CR = 8, 256, 128, 128, 32
HW = H * W
BF = ml_dtypes.bfloat16

_BUILD_CACHE = {}


def _build(with_qkv_bias: bool, with_z_bias: bool, taps: bool = False):
    nc = bacc.Bacc("TRN2", target_bir_lowering=False, debug=False, num_devices=8)
    dt = mybir.dt
    f32, bf16 = dt.float32, dt.bfloat16

    x_d = nc.dram_tensor("x", [C, HW], f32, kind="ExternalInput").ap()
    wkqvT_d = nc.dram_tensor("wkqvT", [C, 96], bf16, kind="ExternalInput").ap()
    wzT_d = nc.dram_tensor("wzT96", [96, C], bf16, kind="ExternalInput").ap()
    mask_d = nc.dram_tensor("mask1", [128, 128], bf16, kind="ExternalInput").ap()
    czo_d = nc.dram_tensor("czo", [2, 512], bf16, kind="ExternalInput").ap()
    ident_d = nc.dram_tensor("identpad", [128, 32], bf16, kind="ExternalInput").ap()
    if with_qkv_bias:
        bvkq_d = nc.dram_tensor("bvkq", [1, 96], bf16, kind="ExternalInput").ap()
    if with_z_bias:
        bzr_d = nc.dram_tensor("bz_row", [1, C], bf16, kind="ExternalInput").ap()

    zscr_r = nc.dram_tensor("zscr_r", [HW], bf16, kind="Internal").ap()
    zscr_c = nc.dram_tensor("zscr_c", [HW], bf16, kind="Internal").ap()
    rscr = nc.dram_tensor("rscr", [HW], bf16, kind="Internal").ap()
    out_d = nc.dram_tensor("out", [C, HW], f32, kind="ExternalOutput").ap()
    if taps:
        d_tB = nc.dram_tensor("d_tB", [32, HW], bf16, kind="ExternalOutput").ap()
        d_vtsr = nc.dram_tensor("d_vtsr", [128, 4 * 32 * 48], bf16,
                                kind="ExternalOutput").ap()
        d_vtsc = nc.dram_tensor("d_vtsc", [128, 128 * 33], bf16,
                                kind="ExternalOutput").ap()
        d_zin = nc.dram_tensor("d_zin", [97, HW], bf16, kind="ExternalOutput").ap()
        d_r = nc.dram_tensor("d_r", [128, 128], bf16, kind="ExternalOutput").ap()

    with tile.TileContext(nc) as tc:
        with (
            tc.tile_pool(name="persist", bufs=1) as pp,
            tc.tile_pool(name="work", bufs=2) as wp,
            tc.tile_pool(name="outw", bufs=2) as op,
            tc.tile_pool(name="psA", bufs=2, space="PSUM") as pA,
            tc.tile_pool(name="psB", bufs=4, space="PSUM") as pB,
        ):
            # ---- persistent SBUF ----
            x_bf = pp.tile([128, 2, HW], bf16)
            # tA rows: K@0, Q@32, V@64 (h-major), ones@96:112
            tA = pp.tile([128, H, W], bf16)
            tB = pp.tile([32, H, W], bf16)    # Q replica
            # zin rows: row-out 0:32, Zr@32, zero hole 33:64,
            # col-out 64:96 (h-major), Zc@96
            zin = pp.tile([97, H, W], bf16)
            vts_row = pp.tile([128, 4, 32, 48], bf16)  # [w, q, hl, c]; ones@c32
            vts_col = pp.tile([128, 128, 33], bf16)    # [h, w, c]; ones@c32
            wkqvT = pp.tile([128, 2, 96], bf16)
            wzT96 = pp.tile([96, C], bf16)
            mask1 = pp.tile([128, 128], bf16)
            ident = pp.tile([128, 32], bf16)

            nc.sync.dma_start(out=wkqvT[:], in_=wkqvT_d.rearrange("(a p) m -> p a m", p=128))
            nc.sync.dma_start(out=wzT96[:], in_=wzT_d)
            nc.sync.dma_start(out=mask1[:], in_=mask_d)
            nc.sync.dma_start(out=ident[:], in_=ident_d)
            if with_qkv_bias or with_z_bias:
                ones_row = pp.tile([1, 512], bf16)
                nc.vector.memset(ones_row[:], 1.0)
            if with_qkv_bias:
                bvkq = pp.tile([1, 96], bf16)
                nc.sync.dma_start(out=bvkq[:], in_=bvkq_d)
            if with_z_bias:
                bz_row = pp.tile([1, C], bf16)
                nc.sync.dma_start(out=bz_row[:], in_=bzr_d)

            # consts via DMA broadcast (DVE memset of [*,16384] is ~14us)
            zrow = czo_d[0:1, :]
            orow = czo_d[1:2, :]
            def _bcast(dst, row, np_, nf):
                bcap = bass.AP(tensor=row.tensor, offset=row.offset,
                               ap=[[0, np_], [0, nf // 512], [1, 512]])
                nc.sync.dma_start(out=dst, in_=bcap)
            _bcast(zin[32:64, :, :], zrow, 32, HW)
            _bcast(tA[96:112, :, :], orow, 16, HW)
            nc.vector.memset(vts_col[:, :, 32:33], 1.0)

            vrow_src = tA[64:112].rearrange("p a b -> p (a b)")  # [48, HW]

            def attn_batch(b8, row_mode, expe_box):
                """Energies+exp for batch b8 (8 stripes), K@0 x Q@0."""
                s0 = b8 * 8
                ps_e = pA.tile([128, 8, 128], f32, tag="pse")
                ksrc, qsrc = tA[0:32], tB[0:32]
                for j in range(8):
                    s = s0 + j
                    if row_mode:
                        lhsT, rhs = ksrc[:, s, :], qsrc[:, s, :]
                    else:
                        lhsT, rhs = ksrc[:, :, s], qsrc[:, :, s]
                    nc.tensor.matmul(ps_e[:, j, :], lhsT, rhs,
                                     start=True, stop=True)
                expe = wp.tile([128, 8, 128], bf16, tag="expe")
                nc.scalar.activation(expe[:], ps_e[:], mybir.ActivationFunctionType.Exp)
                if not row_mode:
                    mk = mask1[:]
                    mb = bass.AP(tensor=mk.tensor, offset=mk.offset,
                                 ap=[list(mk.ap[0]), [0, 8], list(mk.ap[1])])
                    nc.vector.tensor_mul(expe[:], expe[:], mb)
                expe_box[b8] = expe

            zc_view = zin[64:97].rearrange("p h w -> p w h")  # [33, W, H]

            def apply_batch(b8, row_mode, expe_box):
                """V^T @ exp for batch b8. Row -> zin[0:33] via DVE;
                col -> contiguous CAST to zcs then strided hop to
                zin[64:97] h-major (gpsimd/DVE)."""
                s0 = b8 * 8
                expe = expe_box[b8]
                for half in range(2):
                    ps_a = pB.tile([33, 4, 128], f32, tag="psb", name="ps_a")
                    for jj in range(4):
                        j = half * 4 + jj
                        s = s0 + j
                        if row_mode:
                            lhsT = vts_row[:, s // 32, s % 32, 0:33]
                        else:
                            lhsT = vts_col[:, s, :]
                        nc.tensor.matmul(ps_a[:, jj, :], lhsT,
                                         expe[:, j, :], start=True, stop=True)
                    c0 = s0 + half * 4
                    if row_mode:
                        nc.vector.tensor_copy(zin[0:33, c0:c0 + 4, :], ps_a[:])
                    else:
                        zcs = wp.tile([33, 4, 128], bf16, tag="zcs", bufs=3,
                                      name="zcs")
                        if half == 0:
                            nc.vector.tensor_copy(zcs[:], ps_a[:])
                            nc.gpsimd.tensor_copy(zc_view[:, c0:c0 + 4, :], zcs[:])
                        else:
                            nc.scalar.copy(zcs[:], ps_a[:])
                            nc.vector.tensor_copy(zc_view[:, c0:c0 + 4, :], zcs[:])
                expe_box[b8] = None

            def vtsc_group(g):
                """4 PE transposes of V columns 4g..4g+4 -> vts_col."""
                s0 = g * 4
                pv = pB.tile([128, 4, 32], bf16, tag="psb", name="pv")
                for i in range(4):
                    nc.tensor.transpose(pv[:, i, :], tA[64:96, :, s0 + i],
                                        ident[64:96, :], tile_position=(64, 0))
                if g % 2 == 0:
                    nc.vector.tensor_copy(vts_col[:, s0:s0 + 4, 0:32], pv[:])
                else:
                    nc.scalar.copy(vts_col[:, s0:s0 + 4, 0:32], pv[:])

            # ========== P1 + row attention, interleaved by quarters ==========
            expe_box = [None] * 16
            prev_rb = None
            for q in range(4):
                s = q * 4096
                nsub = 4 if q == 0 else 1
                sub = 4096 // nsub
                for si in range(nsub):
                    for half in range(2):
                        s1 = s + si * sub
                        nc.gpsimd.dma_start(
                            out=x_bf[:, half, s1:s1 + sub],
                            in_=x_d[half * 128:(half + 1) * 128, s1:s1 + sub])
                for cp in range(4):  # chunk pairs: shared weight loads
                    ch0 = q * 8 + cp * 2
                    pss = []
                    for ci in range(2):
                        ps = pB.tile([96, 512], f32, tag="psb", name="ps_qkv")
                        pss.append((ps, (ch0 + ci) * 512))
                    for hf in range(2):
                        for ci in range(2):
                            ps, s2 = pss[ci]
                            nc.tensor.matmul(
                                ps[:], wkqvT[:, hf, :], x_bf[:, hf, s2:s2 + 512],
                                start=(hf == 0),
                                stop=(hf == 1) and not with_qkv_bias)
                    if with_qkv_bias:
                        for ci in range(2):
                            ps, s2 = pss[ci]
                            nc.tensor.matmul(ps[:], bvkq[:], ones_row[:],
                                             start=False, stop=True)
                    for ci in range(2):
                        ch = ch0 + ci
                        ps, s2 = pss[ci]
                        h0 = ch * 4
                        ps3 = ps[:].rearrange("p (a b) -> p a b", b=128)
                        nc.scalar.copy(tA[0:96, h0:h0 + 4, :], ps3[0:96])
                # Q replica for this quarter via one SBUF->SBUF DMA
                qh0 = q * 32
                nc.sync.dma_start(out=tB[0:32, qh0:qh0 + 32, :],
                                  in_=tA[32:64, qh0:qh0 + 32, :])
                # V^T stripes for this quarter via DMA XBAR transpose
                nc.sync.dma_start(out=vts_row[:, q],
                                  in_=vrow_src[:, s:s + 4096], transpose=True)
                # row attention for this quarter (software-pipelined)
                for bl in range(4):
                    b8 = q * 4 + bl
                    attn_batch(b8, True, expe_box)
                    if prev_rb is not None:
                        apply_batch(prev_rb, True, expe_box)
                    prev_rb = b8
            apply_batch(prev_rb, True, expe_box)
            # Zr reshape roundtrip can run during the col phase
            nc.sync.dma_start(out=zscr_r.rearrange("(p f) -> p f", p=1),
                              in_=zin[32:33, :, :].rearrange("p a b -> p (a b)"))
            zr2 = wp.tile([128, 128], bf16, tag="zr2", bufs=1)
            nc.sync.dma_start(out=zr2[:], in_=zscr_r.rearrange("(p f) -> p f", p=128))

            # ========== column attention ==========
            for g in range(6):            # vts_col head start (stripes 0-23)
                vtsc_group(g)
            prev = None
            for wb in range(17):
                if wb < 16:
                    attn_batch(wb, False, expe_box)
                    for g in range(2 * wb + 6, min(2 * wb + 8, 32)):
                        vtsc_group(g)
                if prev is not None:
                    apply_batch(prev, False, expe_box)
                prev = wb if wb < 16 else None

            # ========== Z -> r (both Z rows are h-major) ==========
            nc.sync.dma_start(out=zscr_c.rearrange("(p f) -> p f", p=1),
                              in_=zin[96:97, :, :].rearrange("p a b -> p (a b)"))
            zc2 = wp.tile([128, 128], bf16, tag="zc2", bufs=1)
            nc.sync.dma_start(out=zc2[:], in_=zscr_c.rearrange("(p f) -> p f", p=128))
            zs = wp.tile([128, 128], f32, tag="zs", bufs=1)
            nc.vector.tensor_add(zs[:], zr2[:], zc2[:])
            rsq = wp.tile([128, 128], f32, tag="rsq", bufs=1)
            nc.vector.reciprocal(rsq[:], zs[:])
            r_bf = wp.tile([128, 128], bf16, tag="r_bf", bufs=1)
            nc.vector.tensor_copy(r_bf[:], rsq[:])
            nc.sync.dma_start(out=rscr.rearrange("(p f) -> p f", p=128), in_=r_bf[:])

            # ========== P5: prenorm, 96-deep Wz, residual, store ==========
            rb = tA[0:97, :, :]              # r broadcast target (dead K/Q/V)
            src_r = rscr.rearrange("(a b) -> a b", b=128)
            chunks = [(hc * 16, 16) for hc in range(7)] + [(112, 8), (120, 8)]
            for ck, (h0, hn) in enumerate(chunks):
                # broadcast r rows h0:h0+hn to partitions 0-96, then prenorm
                sl = src_r[h0:h0 + hn, :]
                bc = bass.AP(tensor=sl.tensor, offset=sl.offset,
                             ap=[[0, 97]] + list(sl.ap))
                nc.sync.dma_start(out=rb[:, h0:h0 + hn, :], in_=bc)
                if ck % 2 == 0:
                    nc.vector.tensor_mul(zin[:, h0:h0 + hn, :],
                                         zin[:, h0:h0 + hn, :],
                                         rb[:, h0:h0 + hn, :])
                else:
                    nc.gpsimd.tensor_mul(zin[:, h0:h0 + hn, :],
                                         zin[:, h0:h0 + hn, :],
                                         rb[:, h0:h0 + hn, :])
                of = op.tile([128, 2, 16, 128], bf16, tag="of", name="of")
                for wt in range(4):          # w-tiles of 32 cols
                    w0 = wt * 32
                    rhs = zin[0:96, h0:h0 + hn, w0:w0 + 32]
                    for half in range(2):
                        ps_f = pB.tile([128, hn * 32], f32, tag="psb", name="ps_f")
                        wzh = wzT96[:, half * 128:(half + 1) * 128]
                        nc.tensor.matmul(ps_f[:], wzh, rhs,
                                         start=True, stop=not with_z_bias)
                        if with_z_bias:
                            nc.tensor.matmul(
                                ps_f[:], bz_row[:, half * 128:(half + 1) * 128],
                                ones_row[:, 0:hn * 32], start=False, stop=True)
                        x_t = x_bf[:, half, :].rearrange(
                            "p (a b) -> p a b", b=128)[:, h0:h0 + hn, w0:w0 + 32]
                        dst = of[:, half, 0:hn, w0:w0 + 32]
                        psv = ps_f[:].rearrange("p (a b) -> p a b", b=32)
                        if wt % 2 == 0:
                            nc.vector.tensor_add(dst, psv, x_t)
                        else:
                            nc.scalar.copy(dst, psv)
                            nc.gpsimd.tensor_add(dst, dst, x_t)
                # one cast (bf16->f32) DMA for both halves
                od = bass.AP(tensor=out_d.tensor, offset=h0 * 128,
                             ap=[[16384, 128], [128 * 16384, 2], [1, hn * 128]])
                nc.gpsimd.dma_start(
                    out=od, in_=of[:, :, 0:hn, :].rearrange("p a b c -> p a (b c)"))
            if taps:
                nc.sync.dma_start(out=d_tB, in_=tB[0:32].rearrange("p a b -> p (a b)"))
                nc.sync.dma_start(out=d_vtsr,
                                  in_=vts_row[:].rearrange("p a b c -> p (a b c)"))
                nc.sync.dma_start(out=d_vtsc,
                                  in_=vts_col[:].rearrange("p a b -> p (a b)"))
                nc.sync.dma_start(out=d_zin,
                                  in_=zin[:].rearrange("p a b -> p (a b)"))
                nc.sync.dma_start(out=d_r, in_=r_bf[:])
    nc.compile()
    return nc


def _host_prep(Wq, bq, Wk, bk, Wv, bv, Wz, bz):
    wkqvT = np.ascontiguousarray(
        np.concatenate([Wk, Wq, Wv], axis=0).T).astype(BF)          # (256, 96)
    wzT = np.ascontiguousarray(Wz.T).astype(np.float32)              # (32, 256)
    wzT96 = np.zeros((96, C), np.float32)
    wzT96[0:32] = wzT
    wzT96[64:96] = wzT
    wzT96 = wzT96.astype(BF)
    bz_row = np.asarray(bz, np.float32).reshape(1, C).astype(BF)
    eye = np.eye(128, dtype=np.float32)
    mask1 = np.ascontiguousarray(1.0 - eye).astype(BF)
    bvkq = np.concatenate([bk, bq, bv]).reshape(1, 96).astype(BF)
    czo = np.zeros((2, 512), np.float32)
    czo[1] = 1.0
    czo = czo.astype(BF)
    identpad = np.vstack([np.eye(32, dtype=np.float32)] * 4).astype(BF)
    return wkqvT, wzT96, bz_row, mask1, bvkq, czo, identpad


def kernel(x, Wq, bq, Wk, bk, Wv, bv, Wz, bz):
    x = np.asarray(x, np.float32)
    wkqvT, wzT96, bz_row, mask1, bvkq, czo, identpad = _host_prep(
        np.asarray(Wq, np.float32), np.asarray(bq, np.float32),
        np.asarray(Wk, np.float32), np.asarray(bk, np.float32),
        np.asarray(Wv, np.float32), np.asarray(bv, np.float32),
        np.asarray(Wz, np.float32), np.asarray(bz, np.float32))
    with_qkv_bias = bool(np.any(bvkq.astype(np.float32) != 0.0))
    with_z_bias = bool(np.any(bz_row.astype(np.float32) != 0.0))

    key = (with_qkv_bias, with_z_bias)
    if key not in _BUILD_CACHE:
        _BUILD_CACHE[key] = _build(*key)
    nc = _BUILD_CACHE[key]

    in_maps = []
    for b in range(B):
        m = dict(
            x=np.ascontiguousarray(x[b].reshape(C, HW)),
            wkqvT=wkqvT, wzT96=wzT96, mask1=mask1, czo=czo, identpad=identpad,
        )
        if with_qkv_bias:
            m["bvkq"] = bvkq
        if with_z_bias:
            m["bz_row"] = bz_row
        in_maps.append(m)

    res = run_bass_kernel_spmd(nc, in_maps, core_ids=list(range(8)))
    out = np.stack([res.results[b]["out"].reshape(C, H, W) for b in range(B)])
    return out


# revision 24
# speedup vs baseline: 1.4976x; 1.4976x over previous
"""Criss-cross attention block (CCNet) Bass/Tile kernel for Trainium2.

Shapes (hardcoded): B=8, C=256, H=W=128, CR=32. Data-parallel over batch:
core b processes image b. Full inputs in, full output out.

Per-core plan (v2):
  P1   : stream x (f32->bf16 cast in DMA), QKV projections with paired
         weight loads. Single [96,512] ACT evacuation per chunk
         (K@0,Q@32,V@64 contiguous in tA). V@96 + tB replicas via DMA.
  row  : row attention interleaved with P1 by quarters. Energies packed
         2x over PE row-groups (T0: K@0/Q@0, T4: K@32/Q@32), V^T
         transposes packed 2x (T8: V@64, T12: V@96). Row applies
         evacuate via ACT into out_r (h-major, contiguous).
  col  : same packing; applies evacuate via DVE CAST into out_c
         (w-major, contiguous, overlaid on tB[64:97]) -- no strided adds.
  Z    : Z = Zr + Zc^T on-chip (4-block PE transposes), reciprocal,
         r / r^T bulk-broadcast into dead K/Q sbuf (tA[0:32], tA[32:64])
         via DRAM roundtrip.
  P5   : square pixel tiles (16h x 32w): norm_r/norm_c muls, one Wz
         accumulation group per psum tile (rhs_r h-major + rhs_c
         w-major-rearranged), DVE residual add into bf16 staging,
         2 MB cast (bf16->f32) output DMAs.
"""
import sys

sys.path.insert(0, "/opt/trn_rl_repo")

import numpy as np
import ml_dtypes

import concourse.bass as bass
import concourse.mybir as mybir
from concourse import bacc, tile
from concourse.bass_utils import run_bass_kernel_spmd

B, C, H, W, CR = 8, 256, 128, 128, 32
HW = H * W
BF = ml_dtypes.bfloat16

_BUILD_CACHE = {}

# engine for P5 norm multiplies: nc.gpsimd or nc.vector (set in _build)
NORM_ON_GPSIMD = True


def _build(with_qkv_bias: bool, with_z_bias: bool):
    nc = bacc.Bacc("TRN2", target_bir_lowering=False, debug=False, num_devices=8)
    dt = mybir.dt
    f32, bf16 = dt.float32, dt.bfloat16

    x_d = nc.dram_tensor("x", [C, HW], f32, kind="ExternalInput").ap()
    wkqvT_d = nc.dram_tensor("wkqvT", [C, 96], bf16, kind="ExternalInput").ap()
    wzT_d = nc.dram_tensor("wzT", [CR, C], bf16, kind="ExternalInput").ap()
    mask_d = nc.dram_tensor("mask8", [128, 8, 128], bf16, kind="ExternalInput").ap()
    ident_d = nc.dram_tensor("identpad", [128, 32], bf16, kind="ExternalInput").ap()
    if with_qkv_bias:
        bvkq_d = nc.dram_tensor("bvkq", [1, 96], bf16, kind="ExternalInput").ap()
    if with_z_bias:
        bzr_d = nc.dram_tensor("bz_row", [1, C], bf16, kind="ExternalInput").ap()

    zscr_r = nc.dram_tensor("zscr_r", [HW], bf16, kind="Internal").ap()
    zscr_c = nc.dram_tensor("zscr_c", [HW], bf16, kind="Internal").ap()
    rscr_r = nc.dram_tensor("rscr_r", [HW], bf16, kind="Internal").ap()
    rscr_c = nc.dram_tensor("rscr_c", [HW], bf16, kind="Internal").ap()
    out_d = nc.dram_tensor("out", [C, HW], f32, kind="ExternalOutput").ap()

    with tile.TileContext(nc) as tc:
        with (
            tc.tile_pool(name="persist", bufs=1) as pp,
            tc.tile_pool(name="work", bufs=2) as wp,
            tc.tile_pool(name="outw", bufs=3) as op,
            tc.tile_pool(name="rwork", bufs=2) as rp,
            tc.tile_pool(name="psA", bufs=2, space="PSUM") as pA,
            tc.tile_pool(name="psB", bufs=4, space="PSUM") as pB,
        ):
            # ---- persistent SBUF ----
            x_bf = pp.tile([128, 2, HW], bf16)
            # tA rows: K@0, Q@32, V@64, V@96(replica). tB rows: Q@0, K@32,
            # out_c overlay at 64:97.
            tA = pp.tile([128, H, W], bf16)
            tB = pp.tile([128, H, W], bf16)
            out_r = pp.tile([33, H, W], bf16)   # row attn out rows 0-31, Zr row 32
            # col attn out (w-major view [33, W, H]) on tB's dead upper half
            out_c = tB[64:97]
            vts = pp.tile([128, W, 33], bf16)   # V^T stripes (+ones col), shared
            wkqvT = pp.tile([128, 2, 96], bf16)
            wzT = pp.tile([CR, C], bf16)
            mask8 = pp.tile([128, 8, 128], bf16)
            ident = pp.tile([128, 32], bf16)

            nc.sync.dma_start(out=wkqvT[:], in_=wkqvT_d.rearrange("(a p) m -> p a m", p=128))
            nc.sync.dma_start(out=wzT[:], in_=wzT_d)
            nc.sync.dma_start(out=mask8[:], in_=mask_d)
            nc.sync.dma_start(out=ident[:], in_=ident_d)
            if with_qkv_bias or with_z_bias:
                ones_row = pp.tile([1, 512], bf16)
                nc.vector.memset(ones_row[:], 1.0)
            if with_qkv_bias:
                bvkq = pp.tile([1, 96], bf16)
                nc.sync.dma_start(out=bvkq[:], in_=bvkq_d)
            if with_z_bias:
                bz_row = pp.tile([1, C], bf16)
                nc.sync.dma_start(out=bz_row[:], in_=bzr_d)

            nc.vector.memset(vts[:, :, 32:33], 1.0)

            def transpose_batch(b8, row_mode):
                """V^T stripes for 8 rows/cols starting at 8*b8 -> vts.
                Stripes 0-3 via T8 (V@64), 4-7 via T12 (V@96), interleaved."""
                s0 = b8 * 8
                pv_e = pB.tile([128, 4, 32], bf16, tag="psb", name="pv_e")
                pv_o = pB.tile([128, 4, 32], bf16, tag="psb", name="pv_o")
                for i in range(4):
                    for lo in (True, False):
                        j = i if lo else 4 + i
                        base = 64 if lo else 96
                        pv = pv_e if lo else pv_o
                        if row_mode:
                            src = tA[base:base + 32, s0 + j, :]
                        else:
                            src = tA[base:base + 32, :, s0 + j]
                        nc.tensor.transpose(pv[:, i, :], src,
                                            ident[base:base + 32, :],
                                            tile_position=(base, 0))
                nc.vector.tensor_copy(vts[:, s0:s0 + 4, 0:32], pv_e[:])
                nc.vector.tensor_copy(vts[:, s0 + 4:s0 + 8, 0:32], pv_o[:])

            def attn_batch(b8, row_mode, expe_box):
                """Energies+exp for batch b8 (8 stripes).
                Stripes 0-3 on T0 (K@0 x Q@0, psum bank A), 4-7 on T4
                (K@32 x Q@32, bank B); interleaved issue."""
                s0 = b8 * 8
                ps_e = pA.tile([128, 8, 128], f32, tag="pse")
                for i in range(4):
                    for lo in (True, False):
                        j = i if lo else 4 + i
                        s = s0 + j
                        if lo:
                            ksrc, qsrc = tA[0:32], tB[0:32]
                        else:
                            ksrc, qsrc = tB[32:64], tA[32:64]
                        if row_mode:
                            lhsT, rhs = ksrc[:, s, :], qsrc[:, s, :]
                        else:
                            lhsT, rhs = ksrc[:, :, s], qsrc[:, :, s]
                        nc.tensor.matmul(ps_e[:, j, :], lhsT, rhs,
                                         start=True, stop=True)
                expe = wp.tile([128, 8, 128], bf16, tag="expe")
                nc.scalar.activation(expe[:], ps_e[:], mybir.ActivationFunctionType.Exp)
                if not row_mode:
                    nc.vector.tensor_mul(expe[:], expe[:], mask8[:])
                expe_box[b8] = expe

            def apply_batch(b8, row_mode, expe_box):
                """V^T @ exp for batch b8. Row -> out_r via ACT copy,
                col -> out_c via DVE cast. Both contiguous."""
                s0 = b8 * 8
                expe = expe_box[b8]
                for half in range(2):
                    ps_a = pB.tile([33, 4, 128], f32, tag="psb", name="ps_a")
                    for jj in range(4):
                        j = half * 4 + jj
                        nc.tensor.matmul(ps_a[:, jj, :], vts[:, s0 + j, :],
                                         expe[:, j, :], start=True, stop=True)
                    c0 = s0 + half * 4
                    if row_mode:
                        nc.vector.tensor_copy(out_r[:, c0:c0 + 4, :], ps_a[:])
                    else:
                        nc.vector.tensor_copy(out_c[:, c0:c0 + 4, :], ps_a[:])
                expe_box[b8] = None

            # ========== P1 + row attention, interleaved by quarters ==========
            expe_box = [None] * 16
            prev_rb = None
            for q in range(4):
                s = q * 4096
                nsub = 4 if q == 0 else 1
                sub = 4096 // nsub
                for si in range(nsub):
                    for half in range(2):
                        s1 = s + si * sub
                        nc.gpsimd.dma_start(
                            out=x_bf[:, half, s1:s1 + sub],
                            in_=x_d[half * 128:(half + 1) * 128, s1:s1 + sub])
                for cp in range(4):  # chunk pairs: shared weight loads
                    ch0 = q * 8 + cp * 2
                    pss = []
                    for ci in range(2):
                        ps = pB.tile([96, 512], f32, tag="psb", name="ps_qkv")
                        pss.append((ps, (ch0 + ci) * 512))
                    for hf in range(2):
                        for ci in range(2):
                            ps, s2 = pss[ci]
                            nc.tensor.matmul(
                                ps[:], wkqvT[:, hf, :], x_bf[:, hf, s2:s2 + 512],
                                start=(hf == 0),
                                stop=(hf == 1) and not with_qkv_bias)
                    if with_qkv_bias:
                        for ci in range(2):
                            ps, s2 = pss[ci]
                            nc.tensor.matmul(ps[:], bvkq[:], ones_row[:],
                                             start=False, stop=True)
                    for ci in range(2):
                        ps, s2 = pss[ci]
                        h0 = (ch0 + ci) * 4
                        ps3 = ps[:].rearrange("p (a b) -> p a b", b=128)
                        nc.scalar.copy(tA[0:96, h0:h0 + 4, :], ps3[0:96])
                        nc.scalar.copy(tA[96:128, h0:h0 + 4, :], ps3[64:96])
                        nc.vector.tensor_copy(tB[0:32, h0:h0 + 4, :], ps3[32:64])
                        nc.vector.tensor_copy(tB[32:64, h0:h0 + 4, :], ps3[0:32])
                # row attention for this quarter (software-pipelined)
                for bl in range(4):
                    b8 = q * 4 + bl
                    transpose_batch(b8, True)
                    attn_batch(b8, True, expe_box)
                    if prev_rb is not None:
                        apply_batch(prev_rb, True, expe_box)
                    prev_rb = b8
            apply_batch(prev_rb, True, expe_box)
            # Zr reshape + transpose can run during the col phase
            nc.sync.dma_start(out=zscr_r.rearrange("(p f) -> p f", p=1),
                              in_=out_r[32:33, :, :].rearrange("p a b -> p (a b)"))
            zr2 = wp.tile([128, 128], bf16, tag="zr2", bufs=1)
            nc.sync.dma_start(out=zr2[:], in_=zscr_r.rearrange("(p f) -> p f", p=128))

            # ========== column attention ==========
            prev = None
            for wb in range(17):
                if wb < 16:
                    transpose_batch(wb, False)
                    attn_batch(wb, False, expe_box)
                if prev is not None:
                    apply_batch(prev, False, expe_box)
                prev = wb if wb < 16 else None

            # ========== Z -> r, rT (two independent reciprocal chains) ==========
            # zr2 was loaded during the col phase; transpose it now.
            zr2T = wp.tile([128, 128], bf16, tag="zr2T", bufs=1)
            for b4 in range(4):
                p0 = b4 * 32
                zrb = pB.tile([128, 32], bf16, tag="psb", name="zrb")
                nc.tensor.transpose(zrb[:], zr2[p0:p0 + 32, :],
                                    ident[p0:p0 + 32, :], tile_position=(p0, 0))
                nc.vector.tensor_copy(zr2T[:, p0:p0 + 32], zrb[:])
            nc.sync.dma_start(out=zscr_c.rearrange("(p f) -> p f", p=1),
                              in_=out_c[32:33, :, :].rearrange("p a b -> p (a b)"))
            zc2 = wp.tile([128, 128], bf16, tag="zc2", bufs=1)
            nc.sync.dma_start(out=zc2[:], in_=zscr_c.rearrange("(p f) -> p f", p=128))
            # w-major chain: zsT = zc2 + zr2T -> rT  (no transpose after recip)
            zsT = wp.tile([128, 128], f32, tag="zsT", bufs=1)
            nc.vector.tensor_add(zsT[:], zc2[:], zr2T[:])
            rsqT = wp.tile([128, 128], f32, tag="rsqT", bufs=1)
            nc.vector.reciprocal(rsqT[:], zsT[:])
            rT_bf = wp.tile([128, 128], bf16, tag="rT_bf", bufs=1)
            nc.vector.tensor_copy(rT_bf[:], rsqT[:])
            nc.sync.dma_start(out=rscr_c.rearrange("(p f) -> p f", p=128), in_=rT_bf[:])
            # h-major chain: zs = zr2 + zc2T -> r
            zs = wp.tile([128, 128], f32, tag="zs", bufs=1)
            for b4 in range(4):
                p0 = b4 * 32
                zb = pB.tile([128, 32], bf16, tag="psb", name="zb")
                nc.tensor.transpose(zb[:], zc2[p0:p0 + 32, :],
                                    ident[p0:p0 + 32, :], tile_position=(p0, 0))
                nc.vector.tensor_add(zs[:, p0:p0 + 32], zb[:], zr2[:, p0:p0 + 32])
            rsq = wp.tile([128, 128], f32, tag="rsq", bufs=1)
            nc.vector.reciprocal(rsq[:], zs[:])
            r_bf = wp.tile([128, 128], bf16, tag="r_bf", bufs=1)
            nc.vector.tensor_copy(r_bf[:], rsq[:])
            nc.sync.dma_start(out=rscr_r.rearrange("(p f) -> p f", p=128), in_=r_bf[:])
            # bulk broadcasts into dead K/Q sbuf: rb_r = tA[0:32] (h-major),
            # rb_c = tA[32:64] viewed w-major.
            src_r = rscr_r.rearrange("(a b) -> a b", b=128)
            src_c = rscr_c.rearrange("(a b) -> a b", b=128)
            rb_r = tA[0:32, :, :]            # [32, H, W]
            rb_c = tA[64:96, :, :]           # [32, W, H] view (same bytes)
            for hh in range(2):              # split bcasts + in-place prenorm
                sl = src_r[hh * 64:(hh + 1) * 64, :]
                bc = bass.AP(tensor=sl.tensor, offset=sl.offset,
                             ap=[[0, 32]] + list(sl.ap))
                nc.gpsimd.dma_start(out=tA[0:32, hh * 64:(hh + 1) * 64, :], in_=bc)
                nc.vector.tensor_mul(out_r[0:32, hh * 64:(hh + 1) * 64, :],
                                     out_r[0:32, hh * 64:(hh + 1) * 64, :],
                                     rb_r[:, hh * 64:(hh + 1) * 64, :])
            out_cn = tB[0:32]                # normalized col out, base 0 (Q@0 dead)
            for hh in range(2):
                sl = src_c[hh * 64:(hh + 1) * 64, :]
                bc = bass.AP(tensor=sl.tensor, offset=sl.offset,
                             ap=[[0, 32]] + list(sl.ap))
                nc.gpsimd.dma_start(out=tA[64:96, hh * 64:(hh + 1) * 64, :], in_=bc)
                nc.vector.tensor_mul(out_cn[:, hh * 64:(hh + 1) * 64, :],
                                     out_c[0:32, hh * 64:(hh + 1) * 64, :],
                                     rb_c[:, hh * 64:(hh + 1) * 64, :])

            # ========== P5: Wz (direct strided rhs), residual, store ==========
            chunks = [(hc * 16, 16) for hc in range(7)] + [(112, 8), (120, 8)]
            for h0, hn in chunks:            # h-chunks (last one split)
                ofs = []
                for half in range(2):
                    of = op.tile([128, 16, 128], bf16, tag="of", name="of")
                    ofs.append(of)
                for wt in range(4):          # w-tiles of 32 cols
                    w0 = wt * 32
                    rhs_r = out_r[0:32, h0:h0 + hn, w0:w0 + 32]
                    rhs_c = out_cn[:, w0:w0 + 32, h0:h0 + hn].rearrange(
                        "p w h -> p h w")
                    for half in range(2):
                        ps_f = pB.tile([128, hn * 32], f32, tag="psb", name="ps_f")
                        wzh = wzT[:, half * 128:(half + 1) * 128]
                        nc.tensor.matmul(ps_f[:], wzh, rhs_r,
                                         start=True, stop=False)
                        nc.tensor.matmul(ps_f[:], wzh, rhs_c,
                                         start=False, stop=not with_z_bias)
                        if with_z_bias:
                            nc.tensor.matmul(
                                ps_f[:], bz_row[:, half * 128:(half + 1) * 128],
                                ones_row[:, 0:hn * 32], start=False, stop=True)
                        x_t = x_bf[:, half, :].rearrange(
                            "p (a b) -> p a b", b=128)[:, h0:h0 + hn, w0:w0 + 32]
                        dst = ofs[half][:, 0:hn, w0:w0 + 32]
                        psv = ps_f[:].rearrange("p (a b) -> p a b", b=32)
                        if wt % 2 == 0:
                            nc.vector.tensor_add(dst, psv, x_t)
                        else:
                            nc.scalar.copy(dst, psv)
                            nc.gpsimd.tensor_add(dst, dst, x_t)
                for half in range(2):
                    nc.gpsimd.dma_start(
                        out=out_d[half * 128:(half + 1) * 128,
                                  h0 * 128:(h0 + hn) * 128],
                        in_=ofs[half][:, 0:hn, :].rearrange("p a b -> p (a b)"))
    nc.compile()
    return nc


def _host_prep(Wq, bq, Wk, bk, Wv, bv, Wz, bz):
    wkqvT = np.ascontiguousarray(
        np.concatenate([Wk, Wq, Wv], axis=0).T).astype(BF)          # (256, 96)
    wzT = np.ascontiguousarray(Wz.T).astype(BF)                      # (32, 256)
    bz_row = np.asarray(bz, np.float32).reshape(1, C).astype(BF)
    eye = np.eye(128, dtype=np.float32)
    mask8 = np.ascontiguousarray(
        np.broadcast_to((1.0 - eye)[:, None, :], (128, 8, 128))).astype(BF)
    identpad = np.vstack([np.eye(32, dtype=np.float32)] * 4).astype(BF)
    bvkq = np.concatenate([bk, bq, bv]).reshape(1, 96).astype(BF)
    return wkqvT, wzT, bz_row, mask8, identpad, bvkq


def kernel(x, Wq, bq, Wk, bk, Wv, bv, Wz, bz):
    x = np.asarray(x, np.float32)
    wkqvT, wzT, bz_row, mask8, identpad, bvkq = _host_prep(
        np.asarray(Wq, np.float32), np.asarray(bq, np.float32),
        np.asarray(Wk, np.float32), np.asarray(bk, np.float32),
        np.asarray(Wv, np.float32), np.asarray(bv, np.float32),
        np.asarray(Wz, np.float32), np.asarray(bz, np.float32))
    with_qkv_bias = bool(np.any(bvkq.astype(np.float32) != 0.0))
    with_z_bias = bool(np.any(bz_row.astype(np.float32) != 0.0))

    key = (with_qkv_bias, with_z_bias)
    if key not in _BUILD_CACHE:
        _BUILD_CACHE[key] = _build(*key)
    nc = _BUILD_CACHE[key]

    in_maps = []
    for b in range(B):
        m = dict(
            x=np.ascontiguousarray(x[b].reshape(C, HW)),
            wkqvT=wkqvT, wzT=wzT, mask8=mask8, identpad=identpad,
        )
        if with_qkv_bias:
            m["bvkq"] = bvkq
        if with_z_bias:
            m["bz_row"] = bz_row
        in_maps.append(m)

    res = run_bass_kernel_spmd(nc, in_maps, core_ids=list(range(8)))
    out = np.stack([res.results[b]["out"].reshape(C, H, W) for b in range(B)])
    return out



# revision 25
# speedup vs baseline: 1.5192x; 1.0145x over previous
"""Criss-cross attention block (CCNet) Bass/Tile kernel for Trainium2.

Shapes (hardcoded): B=8, C=256, H=W=128, CR=32. Data-parallel over batch:
core b processes image b. Full inputs in, full output out.

Per-core plan (v2):
  P1   : stream x (f32->bf16 cast in DMA), QKV projections with paired
         weight loads. Single [96,512] ACT evacuation per chunk
         (K@0,Q@32,V@64 contiguous in tA). V@96 + tB replicas via DMA.
  row  : row attention interleaved with P1 by quarters. Energies packed
         2x over PE row-groups (T0: K@0/Q@0, T4: K@32/Q@32), V^T
         transposes packed 2x (T8: V@64, T12: V@96). Row applies
         evacuate via ACT into out_r (h-major, contiguous).
  col  : same packing; applies evacuate via DVE CAST into out_c
         (w-major, contiguous, overlaid on tB[64:97]) -- no strided adds.
  Z    : Z = Zr + Zc^T on-chip (4-block PE transposes), reciprocal,
         r / r^T bulk-broadcast into dead K/Q sbuf (tA[0:32], tA[32:64])
         via DRAM roundtrip.
  P5   : square pixel tiles (16h x 32w): norm_r/norm_c muls, one Wz
         accumulation group per psum tile (rhs_r h-major + rhs_c
         w-major-rearranged), DVE residual add into bf16 staging,
         2 MB cast (bf16->f32) output DMAs.
"""
import sys

sys.path.insert(0, "/opt/trn_rl_repo")

import numpy as np
import ml_dtypes

import concourse.bass as bass
import concourse.mybir as mybir
from concourse import bacc, tile
from concourse.bass_utils import run_bass_kernel_spmd

B, C, H, W, CR = 8, 256, 128, 128, 32
HW = H * W
BF = ml_dtypes.bfloat16

_BUILD_CACHE = {}

# engine for P5 norm multiplies: nc.gpsimd or nc.vector (set in _build)
NORM_ON_GPSIMD = True


def _build(with_qkv_bias: bool, with_z_bias: bool):
    nc = bacc.Bacc("TRN2", target_bir_lowering=False, debug=False, num_devices=8)
    dt = mybir.dt
    f32, bf16 = dt.float32, dt.bfloat16

    x_d = nc.dram_tensor("x", [C, HW], f32, kind="ExternalInput").ap()
    wkqvT_d = nc.dram_tensor("wkqvT", [C, 96], bf16, kind="ExternalInput").ap()
    wzT_d = nc.dram_tensor("wzT", [CR, C], bf16, kind="ExternalInput").ap()
    mask_d = nc.dram_tensor("mask8", [128, 8, 128], bf16, kind="ExternalInput").ap()
    ident_d = nc.dram_tensor("identpad", [128, 32], bf16, kind="ExternalInput").ap()
    if with_qkv_bias:
        bvkq_d = nc.dram_tensor("bvkq", [1, 96], bf16, kind="ExternalInput").ap()
    if with_z_bias:
        bzr_d = nc.dram_tensor("bz_row", [1, C], bf16, kind="ExternalInput").ap()

    zscr_r = nc.dram_tensor("zscr_r", [HW], bf16, kind="Internal").ap()
    zscr_c = nc.dram_tensor("zscr_c", [HW], bf16, kind="Internal").ap()
    rscr_r = nc.dram_tensor("rscr_r", [HW], bf16, kind="Internal").ap()
    rscr_c = nc.dram_tensor("rscr_c", [HW], bf16, kind="Internal").ap()
    out_d = nc.dram_tensor("out", [C, HW], f32, kind="ExternalOutput").ap()

    with tile.TileContext(nc) as tc:
        with (
            tc.tile_pool(name="persist", bufs=1) as pp,
            tc.tile_pool(name="work", bufs=2) as wp,
            tc.tile_pool(name="outw", bufs=3) as op,
            tc.tile_pool(name="rwork", bufs=2) as rp,
            tc.tile_pool(name="psA", bufs=2, space="PSUM") as pA,
            tc.tile_pool(name="psB", bufs=4, space="PSUM") as pB,
        ):
            # ---- persistent SBUF ----
            x_bf = pp.tile([128, 2, HW], bf16)
            # tA rows: K@0, Q@32, V@64, V@96(replica). tB rows: Q@0, K@32,
            # out_c overlay at 64:97.
            tA = pp.tile([128, H, W], bf16)
            tB = pp.tile([128, H, W], bf16)
            out_r = pp.tile([33, H, W], bf16)   # row attn out rows 0-31, Zr row 32
            # col attn out (w-major view [33, W, H]) on tB's dead upper half
            out_c = tB[64:97]
            vts = pp.tile([128, W, 33], bf16)   # V^T stripes (+ones col), shared
            wkqvT = pp.tile([128, 2, 96], bf16)
            wzT = pp.tile([CR, C], bf16)
            mask8 = pp.tile([128, 8, 128], bf16)
            ident = pp.tile([128, 32], bf16)

            nc.sync.dma_start(out=wkqvT[:], in_=wkqvT_d.rearrange("(a p) m -> p a m", p=128))
            nc.sync.dma_start(out=wzT[:], in_=wzT_d)
            nc.sync.dma_start(out=mask8[:], in_=mask_d)
            nc.sync.dma_start(out=ident[:], in_=ident_d)
            if with_qkv_bias or with_z_bias:
                ones_row = pp.tile([1, 512], bf16)
                nc.vector.memset(ones_row[:], 1.0)
            if with_qkv_bias:
                bvkq = pp.tile([1, 96], bf16)
                nc.sync.dma_start(out=bvkq[:], in_=bvkq_d)
            if with_z_bias:
                bz_row = pp.tile([1, C], bf16)
                nc.sync.dma_start(out=bz_row[:], in_=bzr_d)

            nc.vector.memset(vts[:, :, 32:33], 1.0)

            def transpose_batch(b8, row_mode):
                """V^T stripes for 8 rows/cols starting at 8*b8 -> vts.
                Stripes 0-3 via T8 (V@64), 4-7 via T12 (V@96), interleaved."""
                s0 = b8 * 8
                pv_e = pB.tile([128, 4, 32], bf16, tag="psb", name="pv_e")
                pv_o = pB.tile([128, 4, 32], bf16, tag="psb", name="pv_o")
                for i in range(4):
                    for lo in (True, False):
                        j = i if lo else 4 + i
                        base = 64 if lo else 96
                        pv = pv_e if lo else pv_o
                        if row_mode:
                            src = tA[base:base + 32, s0 + j, :]
                        else:
                            src = tA[base:base + 32, :, s0 + j]
                        nc.tensor.transpose(pv[:, i, :], src,
                                            ident[base:base + 32, :],
                                            tile_position=(base, 0))
                nc.vector.tensor_copy(vts[:, s0:s0 + 4, 0:32], pv_e[:])
                nc.vector.tensor_copy(vts[:, s0 + 4:s0 + 8, 0:32], pv_o[:])

            def attn_batch(b8, row_mode, expe_box):
                """Energies+exp for batch b8 (8 stripes).
                Stripes 0-3 on T0 (K@0 x Q@0, psum bank A), 4-7 on T4
                (K@32 x Q@32, bank B); interleaved issue."""
                s0 = b8 * 8
                ps_e = pA.tile([128, 8, 128], f32, tag="pse")
                for i in range(4):
                    for lo in (True, False):
                        j = i if lo else 4 + i
                        s = s0 + j
                        if lo:
                            ksrc, qsrc = tA[0:32], tB[0:32]
                        else:
                            ksrc, qsrc = tB[32:64], tA[32:64]
                        if row_mode:
                            lhsT, rhs = ksrc[:, s, :], qsrc[:, s, :]
                        else:
                            lhsT, rhs = ksrc[:, :, s], qsrc[:, :, s]
                        nc.tensor.matmul(ps_e[:, j, :], lhsT, rhs,
                                         start=True, stop=True)
                expe = wp.tile([128, 8, 128], bf16, tag="expe")
                nc.scalar.activation(expe[:], ps_e[:], mybir.ActivationFunctionType.Exp)
                if not row_mode:
                    nc.vector.tensor_mul(expe[:], expe[:], mask8[:])
                expe_box[b8] = expe

            def apply_batch(b8, row_mode, expe_box):
                """V^T @ exp for batch b8. Row -> out_r via ACT copy,
                col -> out_c via DVE cast. Both contiguous."""
                s0 = b8 * 8
                expe = expe_box[b8]
                for half in range(2):
                    ps_a = pB.tile([33, 4, 128], f32, tag="psb", name="ps_a")
                    for jj in range(4):
                        j = half * 4 + jj
                        nc.tensor.matmul(ps_a[:, jj, :], vts[:, s0 + j, :],
                                         expe[:, j, :], start=True, stop=True)
                    c0 = s0 + half * 4
                    if row_mode:
                        nc.vector.tensor_copy(out_r[:, c0:c0 + 4, :], ps_a[:])
                    else:
                        nc.vector.tensor_copy(out_c[:, c0:c0 + 4, :], ps_a[:])
                expe_box[b8] = None

            # ========== P1 + row attention, interleaved by quarters ==========
            expe_box = [None] * 16
            prev_rb = None
            for q in range(4):
                s = q * 4096
                nsub = 4 if q == 0 else 1
                sub = 4096 // nsub
                for si in range(nsub):
                    for half in range(2):
                        s1 = s + si * sub
                        nc.gpsimd.dma_start(
                            out=x_bf[:, half, s1:s1 + sub],
                            in_=x_d[half * 128:(half + 1) * 128, s1:s1 + sub])
                for cp in range(4):  # chunk pairs: shared weight loads
                    ch0 = q * 8 + cp * 2
                    pss = []
                    for ci in range(2):
                        ps = pB.tile([96, 512], f32, tag="psb", name="ps_qkv")
                        pss.append((ps, (ch0 + ci) * 512))
                    for hf in range(2):
                        for ci in range(2):
                            ps, s2 = pss[ci]
                            nc.tensor.matmul(
                                ps[:], wkqvT[:, hf, :], x_bf[:, hf, s2:s2 + 512],
                                start=(hf == 0),
                                stop=(hf == 1) and not with_qkv_bias)
                    if with_qkv_bias:
                        for ci in range(2):
                            ps, s2 = pss[ci]
                            nc.tensor.matmul(ps[:], bvkq[:], ones_row[:],
                                             start=False, stop=True)
                    for ci in range(2):
                        ps, s2 = pss[ci]
                        h0 = (ch0 + ci) * 4
                        ps3 = ps[:].rearrange("p (a b) -> p a b", b=128)
                        nc.scalar.copy(tA[0:96, h0:h0 + 4, :], ps3[0:96])
                        nc.scalar.copy(tA[96:128, h0:h0 + 4, :], ps3[64:96])
                        nc.vector.tensor_copy(tB[0:32, h0:h0 + 4, :], ps3[32:64])
                        nc.vector.tensor_copy(tB[32:64, h0:h0 + 4, :], ps3[0:32])
                # row attention for this quarter (software-pipelined)
                for bl in range(4):
                    b8 = q * 4 + bl
                    transpose_batch(b8, True)
                    attn_batch(b8, True, expe_box)
                    if prev_rb is not None:
                        apply_batch(prev_rb, True, expe_box)
                    prev_rb = b8
            apply_batch(prev_rb, True, expe_box)
            # Zr reshape + transpose can run during the col phase
            nc.sync.dma_start(out=zscr_r.rearrange("(p f) -> p f", p=1),
                              in_=out_r[32:33, :, :].rearrange("p a b -> p (a b)"))
            zr2 = wp.tile([128, 128], bf16, tag="zr2", bufs=1)
            nc.sync.dma_start(out=zr2[:], in_=zscr_r.rearrange("(p f) -> p f", p=128))

            # ========== column attention ==========
            prev = None
            for wb in range(17):
                if wb < 16:
                    transpose_batch(wb, False)
                    attn_batch(wb, False, expe_box)
                if prev is not None:
                    apply_batch(prev, False, expe_box)
                prev = wb if wb < 16 else None

            # ========== Z -> r, rT (two independent reciprocal chains) ==========
            # zr2 was loaded during the col phase; transpose it now.
            zr2T = wp.tile([128, 128], bf16, tag="zr2T", bufs=1)
            for b4 in range(4):
                p0 = b4 * 32
                zrb = pB.tile([128, 32], bf16, tag="psb", name="zrb")
                nc.tensor.transpose(zrb[:], zr2[p0:p0 + 32, :],
                                    ident[p0:p0 + 32, :], tile_position=(p0, 0))
                nc.vector.tensor_copy(zr2T[:, p0:p0 + 32], zrb[:])
            nc.sync.dma_start(out=zscr_c.rearrange("(p f) -> p f", p=1),
                              in_=out_c[32:33, :, :].rearrange("p a b -> p (a b)"))
            zc2 = wp.tile([128, 128], bf16, tag="zc2", bufs=1)
            nc.sync.dma_start(out=zc2[:], in_=zscr_c.rearrange("(p f) -> p f", p=128))
            # w-major chain: zsT = zc2 + zr2T -> rT  (no transpose after recip)
            zsT = wp.tile([128, 128], f32, tag="zsT", bufs=1)
            nc.vector.tensor_add(zsT[:], zc2[:], zr2T[:])
            rsqT = wp.tile([128, 128], f32, tag="rsqT", bufs=1)
            nc.vector.reciprocal(rsqT[:], zsT[:])
            rT_bf = wp.tile([128, 128], bf16, tag="rT_bf", bufs=1)
            nc.vector.tensor_copy(rT_bf[:], rsqT[:])
            nc.sync.dma_start(out=rscr_c.rearrange("(p f) -> p f", p=128), in_=rT_bf[:])
            # h-major chain: zs = zr2 + zc2T -> r
            zs = wp.tile([128, 128], f32, tag="zs", bufs=1)
            for b4 in range(4):
                p0 = b4 * 32
                zb = pB.tile([128, 32], bf16, tag="psb", name="zb")
                nc.tensor.transpose(zb[:], zc2[p0:p0 + 32, :],
                                    ident[p0:p0 + 32, :], tile_position=(p0, 0))
                nc.vector.tensor_add(zs[:, p0:p0 + 32], zb[:], zr2[:, p0:p0 + 32])
            rsq = wp.tile([128, 128], f32, tag="rsq", bufs=1)
            nc.vector.reciprocal(rsq[:], zs[:])
            r_bf = wp.tile([128, 128], bf16, tag="r_bf", bufs=1)
            nc.vector.tensor_copy(r_bf[:], rsq[:])
            nc.sync.dma_start(out=rscr_r.rearrange("(p f) -> p f", p=128), in_=r_bf[:])
            # bulk broadcasts into dead K/Q sbuf: rb_r = tA[0:32] (h-major),
            # rb_c = tA[32:64] viewed w-major.
            src_r = rscr_r.rearrange("(a b) -> a b", b=128)
            src_c = rscr_c.rearrange("(a b) -> a b", b=128)
            rb_r = tA[0:32, :, :]            # [32, H, W]
            rb_c = tA[64:96, :, :]           # [32, W, H] view (same bytes)
            for hh in range(2):              # split bcasts + in-place prenorm
                sl = src_r[hh * 64:(hh + 1) * 64, :]
                bc = bass.AP(tensor=sl.tensor, offset=sl.offset,
                             ap=[[0, 32]] + list(sl.ap))
                nc.sync.dma_start(out=tA[0:32, hh * 64:(hh + 1) * 64, :], in_=bc)
                nc.vector.tensor_mul(out_r[0:32, hh * 64:(hh + 1) * 64, :],
                                     out_r[0:32, hh * 64:(hh + 1) * 64, :],
                                     rb_r[:, hh * 64:(hh + 1) * 64, :])
            out_cn = tB[0:32]                # normalized col out, base 0 (Q@0 dead)
            for hh in range(2):
                sl = src_c[hh * 64:(hh + 1) * 64, :]
                bc = bass.AP(tensor=sl.tensor, offset=sl.offset,
                             ap=[[0, 32]] + list(sl.ap))
                nc.sync.dma_start(out=tA[64:96, hh * 64:(hh + 1) * 64, :], in_=bc)
                nc.vector.tensor_mul(out_cn[:, hh * 64:(hh + 1) * 64, :],
                                     out_c[0:32, hh * 64:(hh + 1) * 64, :],
                                     rb_c[:, hh * 64:(hh + 1) * 64, :])

            # ========== P5: Wz (direct strided rhs), residual, store ==========
            chunks = [(hc * 16, 16) for hc in range(7)] + [(112, 8), (120, 8)]
            for h0, hn in chunks:            # h-chunks (last one split)
                ofs = []
                for half in range(2):
                    of = op.tile([128, 16, 128], bf16, tag="of", name="of")
                    ofs.append(of)
                for wt in range(4):          # w-tiles of 32 cols
                    w0 = wt * 32
                    rhs_r = out_r[0:32, h0:h0 + hn, w0:w0 + 32]
                    rhs_c = out_cn[:, w0:w0 + 32, h0:h0 + hn].rearrange(
                        "p w h -> p h w")
                    for half in range(2):
                        ps_f = pB.tile([128, hn * 32], f32, tag="psb", name="ps_f")
                        wzh = wzT[:, half * 128:(half + 1) * 128]
                        nc.tensor.matmul(ps_f[:], wzh, rhs_r,
                                         start=True, stop=False)
                        nc.tensor.matmul(ps_f[:], wzh, rhs_c,
                                         start=False, stop=not with_z_bias)
                        if with_z_bias:
                            nc.tensor.matmul(
                                ps_f[:], bz_row[:, half * 128:(half + 1) * 128],
                                ones_row[:, 0:hn * 32], start=False, stop=True)
                        x_t = x_bf[:, half, :].rearrange(
                            "p (a b) -> p a b", b=128)[:, h0:h0 + hn, w0:w0 + 32]
                        dst = ofs[half][:, 0:hn, w0:w0 + 32]
                        psv = ps_f[:].rearrange("p (a b) -> p a b", b=32)
                        if wt % 2 == 0:
                            nc.vector.tensor_add(dst, psv, x_t)
                        else:
                            nc.scalar.copy(dst, psv)
                            nc.gpsimd.tensor_add(dst, dst, x_t)
                for half in range(2):
                    nc.gpsimd.dma_start(
                        out=out_d[half * 128:(half + 1) * 128,
                                  h0 * 128:(h0 + hn) * 128],
                        in_=ofs[half][:, 0:hn, :].rearrange("p a b -> p (a b)"))
    nc.compile()
    return nc


def _host_prep(Wq, bq, Wk, bk, Wv, bv, Wz, bz):
    wkqvT = np.ascontiguousarray(
        np.concatenate([Wk, Wq, Wv], axis=0).T).astype(BF)          # (256, 96)
    wzT = np.ascontiguousarray(Wz.T).astype(BF)                      # (32, 256)
    bz_row = np.asarray(bz, np.float32).reshape(1, C).astype(BF)
    eye = np.eye(128, dtype=np.float32)
    mask8 = np.ascontiguousarray(
        np.broadcast_to((1.0 - eye)[:, None, :], (128, 8, 128))).astype(BF)
    identpad = np.vstack([np.eye(32, dtype=np.float32)] * 4).astype(BF)
    bvkq = np.concatenate([bk, bq, bv]).reshape(1, 96).astype(BF)
    return wkqvT, wzT, bz_row, mask8, identpad, bvkq


def kernel(x, Wq, bq, Wk, bk, Wv, bv, Wz, bz):
    x = np.asarray(x, np.float32)
    wkqvT, wzT, bz_row, mask8, identpad, bvkq = _host_prep(
        np.asarray(Wq, np.float32), np.asarray(bq, np.float32),
        np.asarray(Wk, np.float32), np.asarray(bk, np.float32),
        np.asarray(Wv, np.float32), np.asarray(bv, np.float32),
        np.asarray(Wz, np.float32), np.asarray(bz, np.float32))
    with_qkv_bias = bool(np.any(bvkq.astype(np.float32) != 0.0))
    with_z_bias = bool(np.any(bz_row.astype(np.float32) != 0.0))

    key = (with_qkv_bias, with_z_bias)
    if key not in _BUILD_CACHE:
        _BUILD_CACHE[key] = _build(*key)
    nc = _BUILD_CACHE[key]

    in_maps = []
    for b in range(B):
        m = dict(
            x=np.ascontiguousarray(x[b].reshape(C, HW)),
            wkqvT=wkqvT, wzT=wzT, mask8=mask8, identpad=identpad,
        )
        if with_qkv_bias:
            m["bvkq"] = bvkq
        if with_z_bias:
            m["bz_row"] = bz_row
        in_maps.append(m)

    res = run_bass_kernel_spmd(nc, in_maps, core_ids=list(range(8)))
    out = np.stack([res.results[b]["out"].reshape(C, H, W) for b in range(B)])
    return out



# revision 26
# speedup vs baseline: 1.5509x; 1.0208x over previous
"""Criss-cross attention block (CCNet) Bass/Tile kernel for Trainium2.

Shapes (hardcoded): B=8, C=256, H=W=128, CR=32. Data-parallel over batch:
core b processes image b. Full inputs in, full output out.

Per-core plan (v2):
  P1   : stream x (f32->bf16 cast in DMA), QKV projections with paired
         weight loads. Single [96,512] ACT evacuation per chunk
         (K@0,Q@32,V@64 contiguous in tA). V@96 + tB replicas via DMA.
  row  : row attention interleaved with P1 by quarters. Energies packed
         2x over PE row-groups (T0: K@0/Q@0, T4: K@32/Q@32), V^T
         transposes packed 2x (T8: V@64, T12: V@96). Row applies
         evacuate via ACT into out_r (h-major, contiguous).
  col  : same packing; applies evacuate via DVE CAST into out_c
         (w-major, contiguous, overlaid on tB[64:97]) -- no strided adds.
  Z    : Z = Zr + Zc^T on-chip (4-block PE transposes), reciprocal,
         r / r^T bulk-broadcast into dead K/Q sbuf (tA[0:32], tA[32:64])
         via DRAM roundtrip.
  P5   : square pixel tiles (16h x 32w): norm_r/norm_c muls, one Wz
         accumulation group per psum tile (rhs_r h-major + rhs_c
         w-major-rearranged), DVE residual add into bf16 staging,
         2 MB cast (bf16->f32) output DMAs.
"""
import sys

sys.path.insert(0, "/opt/trn_rl_repo")

import numpy as np
import ml_dtypes

import concourse.bass as bass
import concourse.mybir as mybir
from concourse import bacc, tile
from concourse.bass_utils import run_bass_kernel_spmd

B, C, H, W, CR = 8, 256, 128, 128, 32
HW = H * W
BF = ml_dtypes.bfloat16

_BUILD_CACHE = {}

# engine for P5 norm multiplies: nc.gpsimd or nc.vector (set in _build)
NORM_ON_GPSIMD = True


def _build(with_qkv_bias: bool, with_z_bias: bool):
    nc = bacc.Bacc("TRN2", target_bir_lowering=False, debug=False, num_devices=8)
    dt = mybir.dt
    f32, bf16 = dt.float32, dt.bfloat16

    x_d = nc.dram_tensor("x", [C, HW], f32, kind="ExternalInput").ap()
    wkqvT_d = nc.dram_tensor("wkqvT", [C, 96], bf16, kind="ExternalInput").ap()
    wzT_d = nc.dram_tensor("wzT", [CR, C], bf16, kind="ExternalInput").ap()
    mask_d = nc.dram_tensor("mask8", [128, 8, 128], bf16, kind="ExternalInput").ap()
    ident_d = nc.dram_tensor("identpad", [128, 32], bf16, kind="ExternalInput").ap()
    if with_qkv_bias:
        bvkq_d = nc.dram_tensor("bvkq", [1, 96], bf16, kind="ExternalInput").ap()
    if with_z_bias:
        bzr_d = nc.dram_tensor("bz_row", [1, C], bf16, kind="ExternalInput").ap()

    zscr_r = nc.dram_tensor("zscr_r", [HW], bf16, kind="Internal").ap()
    zscr_c = nc.dram_tensor("zscr_c", [HW], bf16, kind="Internal").ap()
    rscr_r = nc.dram_tensor("rscr_r", [HW], bf16, kind="Internal").ap()
    rscr_c = nc.dram_tensor("rscr_c", [HW], bf16, kind="Internal").ap()
    out_d = nc.dram_tensor("out", [C, HW], f32, kind="ExternalOutput").ap()

    with tile.TileContext(nc) as tc:
        with (
            tc.tile_pool(name="persist", bufs=1) as pp,
            tc.tile_pool(name="work", bufs=2) as wp,
            tc.tile_pool(name="outw", bufs=2) as op,
            tc.tile_pool(name="rwork", bufs=2) as rp,
            tc.tile_pool(name="psA", bufs=2, space="PSUM") as pA,
            tc.tile_pool(name="psB", bufs=4, space="PSUM") as pB,
        ):
            # ---- persistent SBUF ----
            x_bf = pp.tile([128, 2, HW], bf16)
            # tA rows: K@0, Q@32, V@64, V@96(replica). tB rows: Q@0, K@32,
            # out_c overlay at 64:97.
            tA = pp.tile([128, H, W], bf16)
            tB = pp.tile([128, H, W], bf16)
            out_r = pp.tile([33, H, W], bf16)   # row attn out rows 0-31, Zr row 32
            # col attn out (w-major view [33, W, H]) on tB's dead upper half
            out_c = tB[64:97]
            vts = pp.tile([128, W, 33], bf16)   # V^T stripes (+ones col), shared
            wkqvT = pp.tile([128, 2, 96], bf16)
            wzT = pp.tile([CR, C], bf16)
            mask8 = pp.tile([128, 8, 128], bf16)
            ident = pp.tile([128, 32], bf16)

            nc.sync.dma_start(out=wkqvT[:], in_=wkqvT_d.rearrange("(a p) m -> p a m", p=128))
            nc.sync.dma_start(out=wzT[:], in_=wzT_d)
            nc.sync.dma_start(out=mask8[:], in_=mask_d)
            nc.sync.dma_start(out=ident[:], in_=ident_d)
            if with_qkv_bias or with_z_bias:
                ones_row = pp.tile([1, 512], bf16)
                nc.vector.memset(ones_row[:], 1.0)
            if with_qkv_bias:
                bvkq = pp.tile([1, 96], bf16)
                nc.sync.dma_start(out=bvkq[:], in_=bvkq_d)
            if with_z_bias:
                bz_row = pp.tile([1, C], bf16)
                nc.sync.dma_start(out=bz_row[:], in_=bzr_d)

            nc.vector.memset(vts[:, :, 32:33], 1.0)

            def transpose_batch(b8, row_mode):
                """V^T stripes for 8 rows/cols starting at 8*b8 -> vts.
                Stripes 0-3 via T8 (V@64), 4-7 via T12 (V@96), interleaved."""
                s0 = b8 * 8
                pv_e = pB.tile([128, 4, 32], bf16, tag="psb", name="pv_e")
                pv_o = pB.tile([128, 4, 32], bf16, tag="psb", name="pv_o")
                for i in range(4):
                    for lo in (True, False):
                        j = i if lo else 4 + i
                        base = 64 if lo else 96
                        pv = pv_e if lo else pv_o
                        if row_mode:
                            src = tA[base:base + 32, s0 + j, :]
                        else:
                            src = tA[base:base + 32, :, s0 + j]
                        nc.tensor.transpose(pv[:, i, :], src,
                                            ident[base:base + 32, :],
                                            tile_position=(base, 0))
                nc.vector.tensor_copy(vts[:, s0:s0 + 4, 0:32], pv_e[:])
                nc.vector.tensor_copy(vts[:, s0 + 4:s0 + 8, 0:32], pv_o[:])

            def attn_batch(b8, row_mode, expe_box):
                """Energies+exp for batch b8 (8 stripes).
                Stripes 0-3 on T0 (K@0 x Q@0, psum bank A), 4-7 on T4
                (K@32 x Q@32, bank B); interleaved issue."""
                s0 = b8 * 8
                ps_e = pA.tile([128, 8, 128], f32, tag="pse")
                for i in range(4):
                    for lo in (True, False):
                        j = i if lo else 4 + i
                        s = s0 + j
                        if lo:
                            ksrc, qsrc = tA[0:32], tB[0:32]
                        else:
                            ksrc, qsrc = tB[32:64], tA[32:64]
                        if row_mode:
                            lhsT, rhs = ksrc[:, s, :], qsrc[:, s, :]
                        else:
                            lhsT, rhs = ksrc[:, :, s], qsrc[:, :, s]
                        nc.tensor.matmul(ps_e[:, j, :], lhsT, rhs,
                                         start=True, stop=True)
                expe = wp.tile([128, 8, 128], bf16, tag="expe")
                nc.scalar.activation(expe[:], ps_e[:], mybir.ActivationFunctionType.Exp)
                if not row_mode:
                    nc.vector.tensor_mul(expe[:], expe[:], mask8[:])
                expe_box[b8] = expe

            def apply_batch(b8, row_mode, expe_box):
                """V^T @ exp for batch b8. Row -> out_r via ACT copy,
                col -> out_c via DVE cast. Both contiguous."""
                s0 = b8 * 8
                expe = expe_box[b8]
                for half in range(2):
                    ps_a = pB.tile([33, 4, 128], f32, tag="psb", name="ps_a")
                    for jj in range(4):
                        j = half * 4 + jj
                        nc.tensor.matmul(ps_a[:, jj, :], vts[:, s0 + j, :],
                                         expe[:, j, :], start=True, stop=True)
                    c0 = s0 + half * 4
                    if row_mode:
                        nc.vector.tensor_copy(out_r[:, c0:c0 + 4, :], ps_a[:])
                    else:
                        nc.vector.tensor_copy(out_c[:, c0:c0 + 4, :], ps_a[:])
                expe_box[b8] = None

            # ========== P1 + row attention, interleaved by quarters ==========
            expe_box = [None] * 16
            prev_rb = None
            for q in range(4):
                s = q * 4096
                nsub = 4 if q == 0 else 1
                sub = 4096 // nsub
                for si in range(nsub):
                    for half in range(2):
                        s1 = s + si * sub
                        nc.gpsimd.dma_start(
                            out=x_bf[:, half, s1:s1 + sub],
                            in_=x_d[half * 128:(half + 1) * 128, s1:s1 + sub])
                for cp in range(4):  # chunk pairs: shared weight loads
                    ch0 = q * 8 + cp * 2
                    pss = []
                    for ci in range(2):
                        ps = pB.tile([96, 512], f32, tag="psb", name="ps_qkv")
                        pss.append((ps, (ch0 + ci) * 512))
                    for hf in range(2):
                        for ci in range(2):
                            ps, s2 = pss[ci]
                            nc.tensor.matmul(
                                ps[:], wkqvT[:, hf, :], x_bf[:, hf, s2:s2 + 512],
                                start=(hf == 0),
                                stop=(hf == 1) and not with_qkv_bias)
                    if with_qkv_bias:
                        for ci in range(2):
                            ps, s2 = pss[ci]
                            nc.tensor.matmul(ps[:], bvkq[:], ones_row[:],
                                             start=False, stop=True)
                    for ci in range(2):
                        ps, s2 = pss[ci]
                        h0 = (ch0 + ci) * 4
                        ps3 = ps[:].rearrange("p (a b) -> p a b", b=128)
                        nc.scalar.copy(tA[0:96, h0:h0 + 4, :], ps3[0:96])
                        nc.scalar.copy(tA[96:128, h0:h0 + 4, :], ps3[64:96])
                        nc.vector.tensor_copy(tB[0:32, h0:h0 + 4, :], ps3[32:64])
                        nc.vector.tensor_copy(tB[32:64, h0:h0 + 4, :], ps3[0:32])
                # row attention for this quarter (software-pipelined)
                for bl in range(4):
                    b8 = q * 4 + bl
                    transpose_batch(b8, True)
                    attn_batch(b8, True, expe_box)
                    if prev_rb is not None:
                        apply_batch(prev_rb, True, expe_box)
                    prev_rb = b8
            apply_batch(prev_rb, True, expe_box)
            # Zr reshape + transpose can run during the col phase
            nc.sync.dma_start(out=zscr_r.rearrange("(p f) -> p f", p=1),
                              in_=out_r[32:33, :, :].rearrange("p a b -> p (a b)"))
            zr2 = wp.tile([128, 128], bf16, tag="zr2", bufs=1)
            nc.sync.dma_start(out=zr2[:], in_=zscr_r.rearrange("(p f) -> p f", p=128))

            # ========== column attention ==========
            prev = None
            for wb in range(17):
                if wb < 16:
                    transpose_batch(wb, False)
                    attn_batch(wb, False, expe_box)
                if prev is not None:
                    apply_batch(prev, False, expe_box)
                prev = wb if wb < 16 else None

            # ========== Z -> r, rT (two independent reciprocal chains) ==========
            # zr2 was loaded during the col phase; transpose it now.
            zr2T = wp.tile([128, 128], bf16, tag="zr2T", bufs=1)
            for b4 in range(4):
                p0 = b4 * 32
                zrb = pB.tile([128, 32], bf16, tag="psb", name="zrb")
                nc.tensor.transpose(zrb[:], zr2[p0:p0 + 32, :],
                                    ident[p0:p0 + 32, :], tile_position=(p0, 0))
                nc.vector.tensor_copy(zr2T[:, p0:p0 + 32], zrb[:])
            nc.sync.dma_start(out=zscr_c.rearrange("(p f) -> p f", p=1),
                              in_=out_c[32:33, :, :].rearrange("p a b -> p (a b)"))
            zc2 = wp.tile([128, 128], bf16, tag="zc2", bufs=1)
            nc.sync.dma_start(out=zc2[:], in_=zscr_c.rearrange("(p f) -> p f", p=128))
            # w-major chain: zsT = zc2 + zr2T -> rT  (no transpose after recip)
            zsT = wp.tile([128, 128], f32, tag="zsT", bufs=1)
            nc.vector.tensor_add(zsT[:], zc2[:], zr2T[:])
            rsqT = wp.tile([128, 128], f32, tag="rsqT", bufs=1)
            nc.vector.reciprocal(rsqT[:], zsT[:])
            rT_bf = wp.tile([128, 128], bf16, tag="rT_bf", bufs=1)
            nc.vector.tensor_copy(rT_bf[:], rsqT[:])
            nc.sync.dma_start(out=rscr_c.rearrange("(p f) -> p f", p=128), in_=rT_bf[:])
            # h-major chain: zs = zr2 + zc2T -> r
            zs = wp.tile([128, 128], f32, tag="zs", bufs=1)
            for b4 in range(4):
                p0 = b4 * 32
                zb = pB.tile([128, 32], bf16, tag="psb", name="zb")
                nc.tensor.transpose(zb[:], zc2[p0:p0 + 32, :],
                                    ident[p0:p0 + 32, :], tile_position=(p0, 0))
                nc.vector.tensor_add(zs[:, p0:p0 + 32], zb[:], zr2[:, p0:p0 + 32])
            rsq = wp.tile([128, 128], f32, tag="rsq", bufs=1)
            nc.vector.reciprocal(rsq[:], zs[:])
            r_bf = wp.tile([128, 128], bf16, tag="r_bf", bufs=1)
            nc.vector.tensor_copy(r_bf[:], rsq[:])
            nc.sync.dma_start(out=rscr_r.rearrange("(p f) -> p f", p=128), in_=r_bf[:])
            # bulk broadcasts into dead K/Q sbuf: rb_r = tA[0:32] (h-major),
            # rb_c = tA[32:64] viewed w-major.
            src_r = rscr_r.rearrange("(a b) -> a b", b=128)
            src_c = rscr_c.rearrange("(a b) -> a b", b=128)
            rb_r = tA[0:32, :, :]            # [32, H, W]
            rb_c = tA[64:96, :, :]           # [32, W, H] view (same bytes)
            for hh in range(2):              # split bcasts + in-place prenorm
                sl = src_r[hh * 64:(hh + 1) * 64, :]
                bc = bass.AP(tensor=sl.tensor, offset=sl.offset,
                             ap=[[0, 32]] + list(sl.ap))
                nc.sync.dma_start(out=tA[0:32, hh * 64:(hh + 1) * 64, :], in_=bc)
                nc.vector.tensor_mul(out_r[0:32, hh * 64:(hh + 1) * 64, :],
                                     out_r[0:32, hh * 64:(hh + 1) * 64, :],
                                     rb_r[:, hh * 64:(hh + 1) * 64, :])
            out_cn = tB[0:32]                # normalized col out, base 0 (Q@0 dead)
            for hh in range(2):
                sl = src_c[hh * 64:(hh + 1) * 64, :]
                bc = bass.AP(tensor=sl.tensor, offset=sl.offset,
                             ap=[[0, 32]] + list(sl.ap))
                nc.sync.dma_start(out=tA[64:96, hh * 64:(hh + 1) * 64, :], in_=bc)
                nc.vector.tensor_mul(out_cn[:, hh * 64:(hh + 1) * 64, :],
                                     out_c[0:32, hh * 64:(hh + 1) * 64, :],
                                     rb_c[:, hh * 64:(hh + 1) * 64, :])

            # ========== P5: Wz (direct strided rhs), residual, store ==========
            chunks = [(hc * 16, 16) for hc in range(7)] + [(112, 8), (120, 8)]
            for h0, hn in chunks:            # h-chunks (last one split)
                of2 = op.tile([128, 2, 16, 128], bf16, tag="of", name="of")
                for wt in range(4):          # w-tiles of 32 cols
                    w0 = wt * 32
                    rhs_r = out_r[0:32, h0:h0 + hn, w0:w0 + 32]
                    rhs_c = out_cn[:, w0:w0 + 32, h0:h0 + hn].rearrange(
                        "p w h -> p h w")
                    for half in range(2):
                        ps_f = pB.tile([128, hn * 32], f32, tag="psb", name="ps_f")
                        wzh = wzT[:, half * 128:(half + 1) * 128]
                        nc.tensor.matmul(ps_f[:], wzh, rhs_r,
                                         start=True, stop=False)
                        nc.tensor.matmul(ps_f[:], wzh, rhs_c,
                                         start=False, stop=not with_z_bias)
                        if with_z_bias:
                            nc.tensor.matmul(
                                ps_f[:], bz_row[:, half * 128:(half + 1) * 128],
                                ones_row[:, 0:hn * 32], start=False, stop=True)
                        x_t = x_bf[:, half, :].rearrange(
                            "p (a b) -> p a b", b=128)[:, h0:h0 + hn, w0:w0 + 32]
                        dst = of2[:, half, 0:hn, w0:w0 + 32]
                        psv = ps_f[:].rearrange("p (a b) -> p a b", b=32)
                        if wt % 2 == 0:
                            nc.vector.tensor_add(dst, psv, x_t)
                        else:
                            nc.scalar.copy(dst, psv)
                            nc.gpsimd.tensor_add(dst, dst, x_t)
                od = bass.AP(tensor=out_d.tensor, offset=h0 * 128,
                             ap=[[16384, 128], [128 * 16384, 2], [1, hn * 128]])
                nc.gpsimd.dma_start(
                    out=od, in_=of2[:, :, 0:hn, :].rearrange("p a b c -> p a (b c)"))
    nc.compile()
    return nc


def _host_prep(Wq, bq, Wk, bk, Wv, bv, Wz, bz):
    wkqvT = np.ascontiguousarray(
        np.concatenate([Wk, Wq, Wv], axis=0).T).astype(BF)          # (256, 96)
    wzT = np.ascontiguousarray(Wz.T).astype(BF)                      # (32, 256)
    bz_row = np.asarray(bz, np.float32).reshape(1, C).astype(BF)
    eye = np.eye(128, dtype=np.float32)
    mask8 = np.ascontiguousarray(
        np.broadcast_to((1.0 - eye)[:, None, :], (128, 8, 128))).astype(BF)
    identpad = np.vstack([np.eye(32, dtype=np.float32)] * 4).astype(BF)
    bvkq = np.concatenate([bk, bq, bv]).reshape(1, 96).astype(BF)
    return wkqvT, wzT, bz_row, mask8, identpad, bvkq


def kernel(x, Wq, bq, Wk, bk, Wv, bv, Wz, bz):
    x = np.asarray(x, np.float32)
    wkqvT, wzT, bz_row, mask8, identpad, bvkq = _host_prep(
        np.asarray(Wq, np.float32), np.asarray(bq, np.float32),
        np.asarray(Wk, np.float32), np.asarray(bk, np.float32),
        np.asarray(Wv, np.float32), np.asarray(bv, np.float32),
        np.asarray(Wz, np.float32), np.asarray(bz, np.float32))
    with_qkv_bias = bool(np.any(bvkq.astype(np.float32) != 0.0))
    with_z_bias = bool(np.any(bz_row.astype(np.float32) != 0.0))

    key = (with_qkv_bias, with_z_bias)
    if key not in _BUILD_CACHE:
        _BUILD_CACHE[key] = _build(*key)
    nc = _BUILD_CACHE[key]

    in_maps = []
    for b in range(B):
        m = dict(
            x=np.ascontiguousarray(x[b].reshape(C, HW)),
            wkqvT=wkqvT, wzT=wzT, mask8=mask8, identpad=identpad,
        )
        if with_qkv_bias:
            m["bvkq"] = bvkq
        if with_z_bias:
            m["bz_row"] = bz_row
        in_maps.append(m)

    res = run_bass_kernel_spmd(nc, in_maps, core_ids=list(range(8)))
    out = np.stack([res.results[b]["out"].reshape(C, H, W) for b in range(B)])
    return out

